# revision 1
# baseline (speedup 1.0000x reference)
"""Trainium2 Bass kernel for nn_CubECLayr: Euler characteristic curves of
sublevel cubical complexes, batch-data-parallel over 8 NeuronCores.

Algorithm (per core, 24 images of 256x256):
  1. k = ceil(x / DT) per pixel (exact integer bin in fp16), via fused
     multiply + magic-number round on the vector engine.
  2. Vertex attribution: every cell (vertex/edge/square) of the cubical
     complex is anchored to its (value, index)-max vertex; the signed count
     of cells anchored at each pixel is an integer delta computed from
     neighbor comparisons in k-space.  Then
         ECC_s = sum_p delta_p * [k_p <= s]
     which is exactly V - E + Sq of the sublevel complex at threshold s.
  3. 32 fused scalar_tensor_tensor passes (compare-multiply-accumulate)
     produce per-partition partial sums; a block-diagonal ones matmul
     reduces partitions -> per-image ECC curves.

Layout: 3 chunks x 8 images; each image owns 16 partitions (16 rows each,
one halo row above/below via SBUF shift-DMAs, pad=1000 at image borders).
"""

import numpy as np

import concourse.bacc as bacc
import concourse.mybir as mybir
from concourse import tile
from concourse.bass_utils import run_bass_kernel_spmd

NCORES = 8
B, C, H, W = 64, 3, 256, 256
IMGS = (B // NCORES) * C          # 24 images per core
CHUNK_IMGS = 8
NCHUNK = IMGS // CHUNK_IMGS       # 3
RB = 16                           # partitions per image
ROWS = H // RB                    # 16 own rows per partition
FD = ROWS * W                     # 4096 own pixels per partition
STEPS = 32
PAD = 1000.0                      # > any real bin; exact in fp16
MAGIC = 8388608.0                 # 2^23
HALF = float(np.float32(0.49999997))
F32 = mybir.dt.float32
F16 = mybir.dt.float16
Op = mybir.AluOpType

_NC_CACHE = {}


def _build_nc():
    nc = bacc.Bacc(None, target_bir_lowering=False)
    x_in = nc.dram_tensor("x", [NCHUNK * 128, FD], F32, kind="ExternalInput")
    bd_in = nc.dram_tensor("bd", [128, NCHUNK * IMGS], F32, kind="ExternalInput")
    out = nc.dram_tensor("out", [IMGS, STEPS], F32, kind="ExternalOutput")

    with tile.TileContext(nc) as tc:
        with (
            tc.tile_pool(name="xp", bufs=2) as xp,
            tc.tile_pool(name="wp", bufs=1) as wp,
            tc.tile_pool(name="cst", bufs=1) as cst,
            tc.tile_pool(name="pp", bufs=1, space="PSUM") as pp,
        ):
            bdt = cst.tile([128, NCHUNK * IMGS], F32)
            nc.sync.dma_start(out=bdt[:], in_=bd_in[:])
            padt = cst.tile([CHUNK_IMGS, W], F16)
            nc.vector.memset(padt[:], PAD)
            psum = pp.tile([IMGS, STEPS], F32)

            for c in range(NCHUNK):
                xt = xp.tile([128, FD], F32, tag="xt")
                nc.sync.dma_start(out=xt[:], in_=x_in[c * 128:(c + 1) * 128, :])

                # --- bins: k = round(x*31 + (0.5 - eps)) == ceil(x/DT) ---
                nc.vector.tensor_scalar(
                    out=xt[:], in0=xt[:], scalar1=31.0, scalar2=HALF,
                    op0=Op.mult, op1=Op.add)
                # kt rows: 0 = top halo, 1..16 own, 17 = bottom halo (flat cols)
                kt = wp.tile([128, (ROWS + 2) * W], F16, tag="kt")
                nc.vector.tensor_scalar(
                    out=kt[:, W:W + FD], in0=xt[:], scalar1=MAGIC, scalar2=-MAGIC,
                    op0=Op.add, op1=Op.add)
                # halo exchange between partitions (same image), pad at borders
                nc.vector.memset(kt[:, 0:W], PAD)
                nc.vector.memset(kt[:, FD + W:FD + 2 * W], PAD)
                nc.gpsimd.dma_start(out=kt[1:128, 0:W], in_=kt[0:127, FD:FD + W])
                nc.gpsimd.dma_start(out=kt[0:127, FD + W:FD + 2 * W],
                                    in_=kt[1:128, W:2 * W])
                ktop = kt[:, 0:W].rearrange("(a b) w -> a b w", b=RB)
                nc.gpsimd.dma_start(out=ktop[:, 0, :], in_=padt[:])
                kbot = kt[:, FD + W:FD + 2 * W].rearrange("(a b) w -> a b w", b=RB)
                nc.gpsimd.dma_start(out=kbot[:, RB - 1, :], in_=padt[:])

                # --- neighbor comparisons (k-space) ---
                # rh[r, j] = [k(r, j+1) >= k(r, j)], own rows, j = 0..254
                # (col 255 crosses rows; harmless, later masked via t zeroing)
                rh = wp.tile([128, FD], F16, tag="rh")
                nc.vector.tensor_tensor(
                    out=rh[:], in0=kt[:, W + 1:W + FD + 1], in1=kt[:, W:W + FD],
                    op=Op.is_ge)
                # rv[t, j] = [k(row t+1) >= k(row t)], t = 0..16 (17 rows)
                rv = wp.tile([128, (ROWS + 1) * W], F16, tag="rv")
                nc.vector.tensor_tensor(
                    out=rv[:], in0=kt[:, W:], in1=kt[:, 0:(ROWS + 1) * W],
                    op=Op.is_ge)
                # khe[r, j] = max(k(r, j), k(r, j+1)), rows 0..17
                khe = wp.tile([128, (ROWS + 2) * W], F16, tag="khe")
                nc.vector.tensor_tensor(
                    out=khe[:, 0:(ROWS + 2) * W - 1],
                    in0=kt[:, 0:(ROWS + 2) * W - 1], in1=kt[:, 1:(ROWS + 2) * W],
                    op=Op.max)
                nc.vector.memset(
                    khe[:, (ROWS + 2) * W - 1:(ROWS + 2) * W], PAD)
                # u[t, j] = [khe(row t+1, j) >= khe(row t, j)], t = 0..16
                ut = wp.tile([128, (ROWS + 1) * W], F16, tag="ut")
                nc.vector.tensor_tensor(
                    out=ut[:], in0=khe[:, W:], in1=khe[:, 0:(ROWS + 1) * W],
                    op=Op.is_ge)
                # Cc[r, j] = u(r) - u(r-1) for own rows r (u rows 1..16 - 0..15)
                cc = wp.tile([128, FD], F16, tag="cc")
                nc.vector.tensor_tensor(
                    out=cc[:], in0=ut[:, W:], in1=ut[:, 0:FD], op=Op.subtract)
                # zero col 255 of each row (cross-row garbage in rh/cc)
                cc3 = cc[:].rearrange("p (r w) -> p r w", w=W)
                nc.vector.memset(cc3[:, :, W - 1:W], 0.0)
                # t = rh * Cc
                tt = wp.tile([128, FD], F16, tag="tt")
                nc.vector.tensor_tensor(out=tt[:], in0=rh[:], in1=cc[:], op=Op.mult)

                # --- delta assembly ---
                # delta = rv(below) - rv(above) + t - shift1(t) - Cc
                dl = wp.tile([128, FD], F16, tag="dl")
                nc.vector.tensor_tensor(
                    out=dl[:], in0=rv[:, W:], in1=rv[:, 0:FD], op=Op.subtract)
                nc.vector.tensor_tensor(out=dl[:], in0=dl[:], in1=tt[:], op=Op.add)
                nc.vector.tensor_tensor(
                    out=dl[:, 1:FD], in0=dl[:, 1:FD], in1=tt[:, 0:FD - 1],
                    op=Op.subtract)
                nc.vector.tensor_tensor(
                    out=dl[:], in0=dl[:], in1=cc[:], op=Op.subtract)

                # --- 32 threshold passes: acc[:, s] = sum(delta * [k <= s]) ---
                acc = wp.tile([128, STEPS], F32, tag="acc")
                wm = wp.tile([128, FD], F16, tag="wm")
                for s in range(STEPS):
                    nc.vector.scalar_tensor_tensor(
                        out=wm[:], in0=kt[:, W:W + FD], scalar=float(s), in1=dl[:],
                        op0=Op.is_le, op1=Op.mult,
                        accum_out=acc[:, s:s + 1])

                # --- partition partials -> per-image curves (PSUM accumulate) ---
                nc.tensor.matmul(
                    psum[:], bdt[:, c * IMGS:(c + 1) * IMGS], acc[:],
                    start=(c == 0), stop=(c == NCHUNK - 1))

            outt = cst.tile([IMGS, STEPS], F32)
            nc.vector.tensor_copy(out=outt[:], in_=psum[:])
            nc.sync.dma_start(out=out[:], in_=outt[:])

    nc.finalize()
    return nc


def _bd_host():
    bd = np.zeros((128, NCHUNK * IMGS), dtype=np.float32)
    for c in range(NCHUNK):
        for p in range(128):
            bd[p, c * IMGS + c * CHUNK_IMGS + p // RB] = 1.0
    return bd


def kernel(x: np.ndarray) -> np.ndarray:
    assert x.shape == (B, C, H, W) and x.dtype == np.float32
    if "nc" not in _NC_CACHE:
        _NC_CACHE["nc"] = _build_nc()
    nc = _NC_CACHE["nc"]

    bd = _bd_host()
    in_maps = []
    for i in range(NCORES):
        shard = x[i * (B // NCORES):(i + 1) * (B // NCORES)]  # (8, 3, 256, 256)
        in_maps.append({
            "x": np.ascontiguousarray(shard).reshape(NCHUNK * 128, FD),
            "bd": bd,
        })
    res = run_bass_kernel_spmd(nc, in_maps, core_ids=list(range(NCORES)))
    parts = [res.results[i]["out"].reshape(B // NCORES, C, STEPS)
             for i in range(NCORES)]
    return np.concatenate(parts, axis=0).reshape(B, C * STEPS).astype(np.float32)


if __name__ == "__main__":
    rng = np.random.default_rng(0)
    x = rng.random((B, C, H, W), dtype=np.float32)
    y = kernel(x)
    print("kernel out", y.shape, y.dtype, y[:2, :6])



# revision 5
# speedup vs baseline: 1.6000x; 1.6000x over previous
"""Trainium2 Bass kernel for nn_CubECLayr: Euler characteristic curves of
sublevel cubical complexes, batch-data-parallel over 8 NeuronCores.

Algorithm (per core, 24 images of 256x256):
  1. k = ceil(x / DT) per pixel (exact integer bin), via fused multiply +
     magic-number round on the vector engine.  k in [1, 31] for x in (0,1).
  2. Vertex attribution: every cell (vertex/edge/square) of the cubical
     complex is anchored to its (value, index)-max vertex; the signed count
     of cells anchored at each pixel is an integer delta computed from
     neighbor comparisons in k-space.  Then ECC_s = sum_p delta_p * [k_p <= s].
  3. Histogram h[b] = sum_p delta_p * [k_p == b] via a CUSTOM DVE op that
     accumulates TWO radix-packed bins per pass:
         accum += delta * ([k==b0] + 2048*[k==b1])
     (16 passes instead of 32 scalar_tensor_tensor passes; each field is
     bounded by max|per-strip-per-bin delta sum| = 161 << 1024, so the
     fp32 accumulator decodes exactly.)
  4. Decode fields on-device (3 tiny vector ops), then two small PE matmuls:
     partition-reduce to per-image histograms and triangular-ones cumsum to
     the final ECC curves (output [32, 24], transposed on host).

Layout: 3 chunks x 8 images; each image owns 16 partitions (16 rows each,
one halo row above/below via SBUF shift-DMAs, pad=1000 at image borders).
"""

import math
from operator import add as _operator_add

import numpy as np

import concourse.bacc as bacc
import concourse.mybir as mybir
from concourse import tile
from concourse.bass_utils import run_bass_kernel_spmd

NCORES = 8
B, C, H, W = 64, 3, 256, 256
IMGS = (B // NCORES) * C          # 24 images per core
CHUNK_IMGS = 8
NCHUNK = IMGS // CHUNK_IMGS       # 3
RB = 16                           # partitions per image
ROWS = H // RB                    # 16 own rows per partition
FD = ROWS * W                     # 4096 own pixels per partition
STEPS = 32
NBINS = 33                        # bins 0..32 (k in [1,31]; 0/32 stay zero)
NPAIR = 16                        # hist passes: pairs (2g+1, 2g+2)
RADIX = 2048.0                    # field packing radix (fields |.| <= 161)
PAD = 1000.0                      # > any real bin; exact in fp16
MAGIC = 8388608.0                 # 2^23
HALF = float(np.float32(0.49999997))
F32 = mybir.dt.float32
F16 = mybir.dt.float16
Op = mybir.AluOpType

_NC_CACHE = {}
_HIST_OP_CACHE = {}


def _get_hist2_op():
    """Register (once per process) the custom DVE op
        out    = in1 * ([in0==s0] + imm2*[in0==s1])
        accum += sum(out)
    and return the DveOp handle."""
    if "op" in _HIST_OP_CACHE:
        return _HIST_OP_CACHE["op"]

    from concourse import dve_ops as dvo
    from concourse.dve_spec import Spec, Src0, Src1, C0, C1, C2, Zero, eq, lower
    from concourse.dve_uop import DveOpSpec

    name = "ECC_HIST2_ANT"

    def _ref(in0, in1, c0, c1, c2):
        a = in0.astype(np.float32)
        d = in1.astype(np.float32)
        body = (d * ((a == c0).astype(np.float32)
                     + c2 * (a == c1).astype(np.float32))).astype(np.float32)
        acc = body.reshape(body.shape[0], -1).sum(axis=-1, keepdims=True)
        return body, acc.astype(np.float32)

    spec = Spec(
        body=Src1 * (eq(Src0, C0) + C2 * eq(Src0, C1)),
        accum=_operator_add,
        accum_init=Zero,
        reference=_ref,
    )

    if name not in dvo._SUB_OPCODE_FOR_NAME:
        row = 1 + len(dvo.OPS)
        assert row < 0x20
        dvo._SUB_OPCODE_FOR_NAME[name] = row
        shas = {}
        for ver in ("v3", "v4"):
            sp = DveOpSpec(name=name, opcode=row, uops=lower(spec, ver=ver),
                           rd1_en=True)
            shas[ver] = sp.sha(ver)
        op = dvo.DveOp(name, spec, subdim=False, uops_sha=shas)
        dvo.OPS.append(op)
        dvo.CUSTOM_DVE_SPECS[name] = spec
    else:
        op = next(o for o in dvo.OPS if o.name == name)

    _HIST_OP_CACHE["op"] = op
    return op


def _build_nc():
    hist2 = _get_hist2_op()

    nc = bacc.Bacc(None, target_bir_lowering=False)
    x_in = nc.dram_tensor("x", [NCHUNK * 128, FD], F32, kind="ExternalInput")
    bd_in = nc.dram_tensor("bd", [128, NCHUNK * IMGS], F32, kind="ExternalInput")
    tri_in = nc.dram_tensor("tri", [NBINS, STEPS], F32, kind="ExternalInput")
    out = nc.dram_tensor("out", [STEPS, IMGS], F32, kind="ExternalOutput")

    with tile.TileContext(nc) as tc:
        with (
            tc.tile_pool(name="xp", bufs=2) as xp,
            tc.tile_pool(name="wp", bufs=1) as wp,
            tc.tile_pool(name="cst", bufs=1) as cst,
            tc.tile_pool(name="pp", bufs=1, space="PSUM") as pp,
            tc.tile_pool(name="pp2", bufs=1, space="PSUM") as pp2,
        ):
            bdt = cst.tile([128, NCHUNK * IMGS], F32)
            nc.sync.dma_start(out=bdt[:], in_=bd_in[:])
            trit = cst.tile([NBINS, STEPS], F32)
            nc.sync.dma_start(out=trit[:], in_=tri_in[:])
            padt = cst.tile([CHUNK_IMGS, W], F16)
            nc.vector.memset(padt[:], PAD)
            # per-chunk decoded histogram; col 0 stays zero, col 33 = scratch
            hist = cst.tile([128, NBINS + 1], F32)
            nc.vector.memset(hist[:], 0.0)
            psum = pp.tile([NBINS, IMGS], F32)

            for c in range(NCHUNK):
                xt = xp.tile([128, FD], F32, tag="xt")
                nc.sync.dma_start(out=xt[:], in_=x_in[c * 128:(c + 1) * 128, :])

                # --- bins: k = round(x*31 + (0.5 - eps)) == ceil(x/DT) ---
                nc.vector.tensor_scalar(
                    out=xt[:], in0=xt[:], scalar1=31.0, scalar2=HALF,
                    op0=Op.mult, op1=Op.add)
                # kt rows: 0 = top halo, 1..16 own, 17 = bottom halo (flat cols)
                kt = wp.tile([128, (ROWS + 2) * W], F16, tag="kt")
                nc.vector.tensor_scalar(
                    out=kt[:, W:W + FD], in0=xt[:], scalar1=MAGIC, scalar2=-MAGIC,
                    op0=Op.add, op1=Op.add)
                # halo exchange between partitions (same image), pad at borders
                nc.vector.memset(kt[:, 0:W], PAD)
                nc.vector.memset(kt[:, FD + W:FD + 2 * W], PAD)
                nc.gpsimd.dma_start(out=kt[1:128, 0:W], in_=kt[0:127, FD:FD + W])
                nc.gpsimd.dma_start(out=kt[0:127, FD + W:FD + 2 * W],
                                    in_=kt[1:128, W:2 * W])
                ktop = kt[:, 0:W].rearrange("(a b) w -> a b w", b=RB)
                nc.gpsimd.dma_start(out=ktop[:, 0, :], in_=padt[:])
                kbot = kt[:, FD + W:FD + 2 * W].rearrange("(a b) w -> a b w", b=RB)
                nc.gpsimd.dma_start(out=kbot[:, RB - 1, :], in_=padt[:])

                # --- neighbor comparisons (k-space) ---
                # rh[r, j] = [k(r, j+1) >= k(r, j)], own rows, j = 0..254
                # (col 255 crosses rows; harmless, later masked via t zeroing)
                rh = wp.tile([128, FD], F16, tag="rh")
                nc.vector.tensor_tensor(
                    out=rh[:], in0=kt[:, W + 1:W + FD + 1], in1=kt[:, W:W + FD],
                    op=Op.is_ge)
                # rv[t, j] = [k(row t+1) >= k(row t)], t = 0..16 (17 rows)
                rv = wp.tile([128, (ROWS + 1) * W], F16, tag="rv")
                nc.vector.tensor_tensor(
                    out=rv[:], in0=kt[:, W:], in1=kt[:, 0:(ROWS + 1) * W],
                    op=Op.is_ge)
                # khe[r, j] = max(k(r, j), k(r, j+1)), rows 0..17
                khe = wp.tile([128, (ROWS + 2) * W], F16, tag="khe")
                nc.vector.tensor_tensor(
                    out=khe[:, 0:(ROWS + 2) * W - 1],
                    in0=kt[:, 0:(ROWS + 2) * W - 1], in1=kt[:, 1:(ROWS + 2) * W],
                    op=Op.max)
                nc.vector.memset(
                    khe[:, (ROWS + 2) * W - 1:(ROWS + 2) * W], PAD)
                # u[t, j] = [khe(row t+1, j) >= khe(row t, j)], t = 0..16
                ut = wp.tile([128, (ROWS + 1) * W], F16, tag="ut")
                nc.vector.tensor_tensor(
                    out=ut[:], in0=khe[:, W:], in1=khe[:, 0:(ROWS + 1) * W],
                    op=Op.is_ge)
                # Cc[r, j] = u(r) - u(r-1) for own rows r (u rows 1..16 - 0..15)
                cc = wp.tile([128, FD], F16, tag="cc")
                nc.vector.tensor_tensor(
                    out=cc[:], in0=ut[:, W:], in1=ut[:, 0:FD], op=Op.subtract)
                # zero col 255 of each row (cross-row garbage in rh/cc)
                cc3 = cc[:].rearrange("p (r w) -> p r w", w=W)
                nc.vector.memset(cc3[:, :, W - 1:W], 0.0)
                # t = rh * Cc
                tt = wp.tile([128, FD], F16, tag="tt")
                nc.vector.tensor_tensor(out=tt[:], in0=rh[:], in1=cc[:], op=Op.mult)

                # --- delta assembly ---
                # delta = rv(below) - rv(above) + t - shift1(t) - Cc
                dl = wp.tile([128, FD], F16, tag="dl")
                nc.vector.tensor_tensor(
                    out=dl[:], in0=rv[:, W:], in1=rv[:, 0:FD], op=Op.subtract)
                nc.vector.tensor_tensor(out=dl[:], in0=dl[:], in1=tt[:], op=Op.add)
                nc.vector.tensor_tensor(
                    out=dl[:, 1:FD], in0=dl[:, 1:FD], in1=tt[:, 0:FD - 1],
                    op=Op.subtract)
                nc.vector.tensor_tensor(
                    out=dl[:], in0=dl[:], in1=cc[:], op=Op.subtract)

                # --- 16 packed-histogram passes:
                #     acc[:, g] = sum(delta * ([k==2g+1] + RADIX*[k==2g+2])) ---
                acc = wp.tile([128, NPAIR], F32, tag="acc")
                wm = wp.tile([128, FD], F16, tag="wm")
                for g in range(NPAIR):
                    nc.vector._custom_dve(
                        hist2,
                        out=wm[:],
                        in0=kt[:, W:W + FD],
                        in1=dl[:],
                        s0=float(2 * g + 1),
                        s1=float(2 * g + 2),
                        imm2=RADIX,
                        accum_out=acc[:, g:g + 1],
                    )

                # --- decode packed fields into hist[:, 1..32] ---
                # hi = round(acc / RADIX); lo = acc - RADIX*hi
                dec = wp.tile([128, NPAIR], F32, tag="dec")
                nc.vector.tensor_scalar(
                    out=dec[:], in0=acc[:], scalar1=1.0 / RADIX, scalar2=MAGIC,
                    op0=Op.mult, op1=Op.add)
                # hi -> even bins 2,4,...,32
                hist_hi = hist[:, 2:NBINS + 1].rearrange("p (g two) -> p g two", two=2)
                nc.vector.tensor_scalar(
                    out=hist_hi[:, :, 0], in0=dec[:], scalar1=-MAGIC, scalar2=0.0,
                    op0=Op.add, op1=Op.add)
                # lo = acc - RADIX*hi -> odd bins 1,3,...,31
                hist_lo = hist[:, 1:NBINS].rearrange("p (g two) -> p g two", two=2)
                nc.vector.scalar_tensor_tensor(
                    out=hist_lo[:, :, 0], in0=hist_hi[:, :, 0], scalar=-RADIX,
                    in1=acc[:], op0=Op.mult, op1=Op.add)

                # --- partition partials -> per-bin-per-image (PSUM accumulate) ---
                nc.tensor.matmul(
                    psum[:], hist[:, 0:NBINS], bdt[:, c * IMGS:(c + 1) * IMGS],
                    start=(c == 0), stop=(c == NCHUNK - 1))

            # --- cumulative sum over bins via triangular-ones matmul ---
            h2 = cst.tile([NBINS, IMGS], F32)
            nc.vector.tensor_copy(out=h2[:], in_=psum[:])
            psum3 = pp2.tile([STEPS, IMGS], F32)
            nc.tensor.matmul(psum3[:], trit[:], h2[:], start=True, stop=True)
            outt = cst.tile([STEPS, IMGS], F32)
            nc.vector.tensor_copy(out=outt[:], in_=psum3[:])
            nc.sync.dma_start(out=out[:], in_=outt[:])

    nc.finalize()
    return nc


def _bd_host():
    bd = np.zeros((128, NCHUNK * IMGS), dtype=np.float32)
    for c in range(NCHUNK):
        for p in range(128):
            bd[p, c * IMGS + c * CHUNK_IMGS + p // RB] = 1.0
    return bd


def _tri_host():
    # tri[b, s] = 1 iff b <= s  (cumulative histogram)
    b = np.arange(NBINS)[:, None]
    s = np.arange(STEPS)[None, :]
    return (b <= s).astype(np.float32)


def kernel(x: np.ndarray) -> np.ndarray:
    assert x.shape == (B, C, H, W) and x.dtype == np.float32
    if "nc" not in _NC_CACHE:
        _NC_CACHE["nc"] = _build_nc()
    nc = _NC_CACHE["nc"]

    bd = _bd_host()
    tri = _tri_host()
    in_maps = []
    for i in range(NCORES):
        shard = x[i * (B // NCORES):(i + 1) * (B // NCORES)]  # (8, 3, 256, 256)
        in_maps.append({
            "x": np.ascontiguousarray(shard).reshape(NCHUNK * 128, FD),
            "bd": bd,
            "tri": tri,
        })
    res = run_bass_kernel_spmd(nc, in_maps, core_ids=list(range(NCORES)))
    parts = [res.results[i]["out"].T.reshape(B // NCORES, C, STEPS)
             for i in range(NCORES)]
    return np.concatenate(parts, axis=0).reshape(B, C * STEPS).astype(np.float32)


if __name__ == "__main__":
    rng = np.random.default_rng(0)
    x = rng.random((B, C, H, W), dtype=np.float32)
    y = kernel(x)
    print("kernel out", y.shape, y.dtype, y[:2, :6])


# revision 10
# speedup vs baseline: 1.7281x; 1.0800x over previous
"""Trainium2 Bass kernel for nn_CubECLayr: Euler characteristic curves of
sublevel cubical complexes, batch-data-parallel over 8 NeuronCores.

Algorithm (per core, 24 images of 256x256):
  1. k = ceil(x / DT) per pixel (exact integer bin), via fused multiply +
     magic-number round on the vector engine.  k in [1, 31] for x in (0,1).
  2. Vertex attribution: every cell (vertex/edge/square) of the cubical
     complex is anchored to its (value, index)-max vertex; the signed count
     of cells anchored at each pixel is an integer delta computed from
     neighbor comparisons in k-space.  Then ECC_s = sum_p delta_p * [k_p <= s].
  3. Histogram h[b] = sum_p delta_p * [k_p == b] via a CUSTOM DVE op that
     accumulates TWO radix-packed bins per pass:
         accum += delta * ([k==b0] + 2048*[k==b1])
     15 passes cover bins 1..30; bin 31 = sum(delta) - sum(bins 1..30),
     where sum(delta) rides the accumulator of the last delta-assembly op
     (tensor_tensor_reduce).  Fields are bounded by max|per-strip per-bin
     delta sum| = 161 << 1024, so the fp32 accumulator decodes exactly.
  4. Decode fields on-device (tiny vector ops), then two small PE matmuls:
     partition-reduce to per-image histograms and triangular-ones cumsum to
     the final ECC curves (output [32, 24], transposed on host).

Pipelining: kt is double-buffered; chunk c+1's bin-conversion and halo
shift-DMAs are issued between chunk c's delta assembly and its histogram
passes, so the halo DMA latency hides under the 15 histogram passes.

Layout: 3 chunks x 8 images; each image owns 16 partitions (16 rows each,
one halo row above/below via partition-shift SBUF DMAs; image-boundary
halo rows are PAD via two 8-partition memsets).
"""

from operator import add as _operator_add

import numpy as np

import concourse.bacc as bacc
import concourse.mybir as mybir
from concourse import tile
from concourse.bass_utils import run_bass_kernel_spmd

NCORES = 8
B, C, H, W = 64, 3, 256, 256
IMGS = (B // NCORES) * C          # 24 images per core
CHUNK_IMGS = 8
NCHUNK = IMGS // CHUNK_IMGS       # 3
RB = 16                           # partitions per image
ROWS = H // RB                    # 16 own rows per partition
FD = ROWS * W                     # 4096 own pixels per partition
STEPS = 32
NBINS = 33                        # bins 0..32 (k in [1,31]; 0/32 stay zero)
NPAIR = 15                        # hist passes: pairs (2g+1, 2g+2), bins 1..30
RADIX = 2048.0                    # field packing radix (fields |.| <= 161)
PAD = 1000.0                      # > any real bin; exact in fp16
MAGIC = 8388608.0                 # 2^23
HALF = float(np.float32(0.49999997))
F32 = mybir.dt.float32
F16 = mybir.dt.float16
Op = mybir.AluOpType
Ax = mybir.AxisListType

_NC_CACHE = {}
_HIST_OP_CACHE = {}


def _get_hist2_op():
    """Register (once per process) the custom DVE op
        out    = in1 * ([in0==s0] + imm2*[in0==s1])
        accum += sum(out)
    and return the DveOp handle."""
    if "op" in _HIST_OP_CACHE:
        return _HIST_OP_CACHE["op"]

    from concourse import dve_ops as dvo
    from concourse.dve_spec import Spec, Src0, Src1, C0, C1, C2, Zero, eq, lower
    from concourse.dve_uop import DveOpSpec

    name = "ECC_HIST2_ANT"

    def _ref(in0, in1, c0, c1, c2):
        a = in0.astype(np.float32)
        d = in1.astype(np.float32)
        body = (d * ((a == c0).astype(np.float32)
                     + c2 * (a == c1).astype(np.float32))).astype(np.float32)
        acc = body.reshape(body.shape[0], -1).sum(axis=-1, keepdims=True)
        return body, acc.astype(np.float32)

    spec = Spec(
        body=Src1 * (eq(Src0, C0) + C2 * eq(Src0, C1)),
        accum=_operator_add,
        accum_init=Zero,
        reference=_ref,
    )

    if name not in dvo._SUB_OPCODE_FOR_NAME:
        row = 1 + len(dvo.OPS)
        assert row < 0x20
        dvo._SUB_OPCODE_FOR_NAME[name] = row
        shas = {}
        for ver in ("v3", "v4"):
            sp = DveOpSpec(name=name, opcode=row, uops=lower(spec, ver=ver),
                           rd1_en=True)
            shas[ver] = sp.sha(ver)
        op = dvo.DveOp(name, spec, subdim=False, uops_sha=shas)
        dvo.OPS.append(op)
        dvo.CUSTOM_DVE_SPECS[name] = spec
    else:
        op = next(o for o in dvo.OPS if o.name == name)

    _HIST_OP_CACHE["op"] = op
    return op


def _build_nc():
    hist2 = _get_hist2_op()

    nc = bacc.Bacc(None, target_bir_lowering=False)
    x_in = nc.dram_tensor("x", [NCHUNK * 128, FD], F32, kind="ExternalInput")
    bd_in = nc.dram_tensor("bd", [128, NCHUNK * IMGS], F32, kind="ExternalInput")
    tri_in = nc.dram_tensor("tri", [NBINS, STEPS], F32, kind="ExternalInput")
    out = nc.dram_tensor("out", [STEPS, IMGS], F32, kind="ExternalOutput")

    with tile.TileContext(nc) as tc:
        with (
            tc.tile_pool(name="xp", bufs=2) as xp,
            tc.tile_pool(name="kp", bufs=2) as kp,
            tc.tile_pool(name="wp", bufs=1) as wp,
            tc.tile_pool(name="cst", bufs=1) as cst,
            tc.tile_pool(name="pp", bufs=1, space="PSUM") as pp,
            tc.tile_pool(name="pp2", bufs=1, space="PSUM") as pp2,
        ):
            bdt = cst.tile([128, NCHUNK * IMGS], F32)
            nc.sync.dma_start(out=bdt[:], in_=bd_in[:])
            trit = cst.tile([NBINS, STEPS], F32)
            nc.sync.dma_start(out=trit[:], in_=tri_in[:])
            padt = cst.tile([CHUNK_IMGS, W], F16)
            nc.vector.memset(padt[:], PAD)
            # per-chunk decoded histogram [128, 33]; cols 0 and 32 stay zero
            hist = cst.tile([128, NBINS], F32)
            nc.vector.memset(hist[:], 0.0)
            psum = pp.tile([NBINS, IMGS], F32)

            def conv_and_halo(c, split):
                """DMA chunk c, convert to f16 bins kt (rows 1..16 at cols
                W..W+FD), fill halo rows: partition-shift DMAs for interior
                partitions, PAD memsets at image boundaries."""
                kt = kp.tile([128, (ROWS + 2) * W], F16, tag="kt")
                xt = xp.tile([128, FD], F32, tag="xt")
                nfd = FD // split
                for h in range(split):
                    sl = slice(h * nfd, (h + 1) * nfd)
                    ksl = slice(W + h * nfd, W + (h + 1) * nfd)
                    nc.sync.dma_start(
                        out=xt[:, sl], in_=x_in[c * 128:(c + 1) * 128, sl])
                    nc.vector.tensor_scalar(
                        out=xt[:, sl], in0=xt[:, sl], scalar1=31.0, scalar2=HALF,
                        op0=Op.mult, op1=Op.add)
                    nc.vector.tensor_scalar(
                        out=kt[:, ksl], in0=xt[:, sl], scalar1=MAGIC,
                        scalar2=-MAGIC, op0=Op.add, op1=Op.add)
                # halo rows: top halo of partition p <- last own row of p-1,
                # bottom halo of p <- first own row of p+1; image-boundary
                # halo rows get PAD.  Shift-DMAs on gpsimd, pad-DMAs on sync.
                nc.vector.memset(kt[:, 0:W], PAD)
                nc.vector.memset(kt[:, FD + W:FD + 2 * W], PAD)
                nc.gpsimd.dma_start(out=kt[1:128, 0:W], in_=kt[0:127, FD:FD + W])
                nc.gpsimd.dma_start(out=kt[0:127, FD + W:FD + 2 * W],
                                    in_=kt[1:128, W:2 * W])
                ktop = kt[:, 0:W].rearrange("(a b) w -> a b w", b=RB)
                nc.gpsimd.dma_start(out=ktop[:, 0, :], in_=padt[:])
                kbot = kt[:, FD + W:FD + 2 * W].rearrange("(a b) w -> a b w", b=RB)
                nc.gpsimd.dma_start(out=kbot[:, RB - 1, :], in_=padt[:])
                return kt

            kts = [None] * NCHUNK
            kts[0] = conv_and_halo(0, split=2)

            for c in range(NCHUNK):
                kt = kts[c]

                # --- neighbor comparisons (k-space) ---
                # rh[r, j] = [k(r, j+1) >= k(r, j)], own rows, j = 0..254
                # (col 255 crosses rows; harmless, later masked via t zeroing)
                rh = wp.tile([128, FD], F16, tag="rh")
                nc.vector.tensor_tensor(
                    out=rh[:], in0=kt[:, W + 1:W + FD + 1], in1=kt[:, W:W + FD],
                    op=Op.is_ge)
                # rv[t, j] = [k(row t+1) >= k(row t)], t = 0..16 (17 rows)
                rv = wp.tile([128, (ROWS + 1) * W], F16, tag="rv")
                nc.vector.tensor_tensor(
                    out=rv[:], in0=kt[:, W:], in1=kt[:, 0:(ROWS + 1) * W],
                    op=Op.is_ge)
                # khe[r, j] = max(k(r, j), k(r, j+1)), rows 0..17
                khe = wp.tile([128, (ROWS + 2) * W], F16, tag="khe")
                nc.vector.tensor_tensor(
                    out=khe[:, 0:(ROWS + 2) * W - 1],
                    in0=kt[:, 0:(ROWS + 2) * W - 1], in1=kt[:, 1:(ROWS + 2) * W],
                    op=Op.max)
                nc.vector.memset(
                    khe[:, (ROWS + 2) * W - 1:(ROWS + 2) * W], PAD)
                # u[t, j] = [khe(row t+1, j) >= khe(row t, j)], t = 0..16
                ut = wp.tile([128, (ROWS + 1) * W], F16, tag="ut")
                nc.vector.tensor_tensor(
                    out=ut[:], in0=khe[:, W:], in1=khe[:, 0:(ROWS + 1) * W],
                    op=Op.is_ge)
                # Cc[r, j] = u(r) - u(r-1) for own rows r (u rows 1..16 - 0..15)
                cc = wp.tile([128, FD], F16, tag="cc")
                nc.vector.tensor_tensor(
                    out=cc[:], in0=ut[:, W:], in1=ut[:, 0:FD], op=Op.subtract)
                # zero col 255 of each row (cross-row garbage in rh/cc)
                cc3 = cc[:].rearrange("p (r w) -> p r w", w=W)
                nc.vector.memset(cc3[:, :, W - 1:W], 0.0)
                # t = rh * Cc
                tt = wp.tile([128, FD], F16, tag="tt")
                nc.vector.tensor_tensor(out=tt[:], in0=rh[:], in1=cc[:], op=Op.mult)

                # --- delta assembly ---
                # delta = rv(below) - rv(above) + t - shift1(t) - Cc
                dl = wp.tile([128, FD], F16, tag="dl")
                nc.vector.tensor_tensor(
                    out=dl[:], in0=rv[:, W:], in1=rv[:, 0:FD], op=Op.subtract)
                nc.vector.tensor_tensor(out=dl[:], in0=dl[:], in1=tt[:], op=Op.add)
                nc.vector.tensor_tensor(
                    out=dl[:, 1:FD], in0=dl[:, 1:FD], in1=tt[:, 0:FD - 1],
                    op=Op.subtract)
                # final: dl -= Cc, with accumulator tot = sum(delta)
                tot = wp.tile([128, 1], F32, tag="tot")
                nc.vector.scalar_tensor_tensor(
                    out=dl[:], in0=dl[:], scalar=0.0, in1=cc[:],
                    op0=Op.add, op1=Op.subtract, accum_out=tot[:])

                # pipeline: issue next chunk's convert + halo DMAs now, so the
                # DMAs run under this chunk's histogram passes
                if c + 1 < NCHUNK:
                    kts[c + 1] = conv_and_halo(c + 1, split=1)

                # --- 15 packed-histogram passes:
                #     acc[:, g] = sum(delta * ([k==2g+1] + RADIX*[k==2g+2])) ---
                acc = wp.tile([128, NPAIR], F32, tag="acc")
                wm = wp.tile([128, FD], F16, tag="wm")
                for g in range(NPAIR):
                    nc.vector._custom_dve(
                        hist2,
                        out=wm[:],
                        in0=kt[:, W:W + FD],
                        in1=dl[:],
                        s0=float(2 * g + 1),
                        s1=float(2 * g + 2),
                        imm2=RADIX,
                        accum_out=acc[:, g:g + 1],
                    )

                # --- decode packed fields into hist[:, 1..31] ---
                # hi = round(acc / RADIX); lo = acc - RADIX*hi
                dec = wp.tile([128, NPAIR], F32, tag="dec")
                nc.vector.tensor_scalar(
                    out=dec[:], in0=acc[:], scalar1=1.0 / RADIX, scalar2=MAGIC,
                    op0=Op.mult, op1=Op.add)
                # hi -> even bins 2,4,...,30
                hist_hi = hist[:, 2:STEPS].rearrange("p (g two) -> p g two", two=2)
                nc.vector.tensor_scalar(
                    out=hist_hi[:, :, 0], in0=dec[:], scalar1=-MAGIC, scalar2=0.0,
                    op0=Op.add, op1=Op.add)
                # lo = acc - RADIX*hi -> odd bins 1,3,...,29
                hist_lo = hist[:, 1:STEPS - 1].rearrange("p (g two) -> p g two", two=2)
                nc.vector.scalar_tensor_tensor(
                    out=hist_lo[:, :, 0], in0=hist_hi[:, :, 0], scalar=-RADIX,
                    in1=acc[:], op0=Op.mult, op1=Op.add)
                # bin 31 = tot - sum(bins 1..30)
                s30 = wp.tile([128, 1], F32, tag="s30")
                nc.vector.tensor_reduce(
                    out=s30[:], in_=hist[:, 1:STEPS - 1], axis=Ax.X, op=Op.add)
                nc.vector.tensor_tensor(
                    out=hist[:, STEPS - 1:STEPS], in0=tot[:], in1=s30[:],
                    op=Op.subtract)

                # --- partition partials -> per-bin-per-image (PSUM accumulate) ---
                nc.tensor.matmul(
                    psum[:], hist[:], bdt[:, c * IMGS:(c + 1) * IMGS],
                    start=(c == 0), stop=(c == NCHUNK - 1))

            # --- cumulative sum over bins via triangular-ones matmul ---
            h2 = cst.tile([NBINS, IMGS], F32)
            nc.vector.tensor_copy(out=h2[:], in_=psum[:])
            psum3 = pp2.tile([STEPS, IMGS], F32)
            nc.tensor.matmul(psum3[:], trit[:], h2[:], start=True, stop=True)
            outt = cst.tile([STEPS, IMGS], F32)
            nc.vector.tensor_copy(out=outt[:], in_=psum3[:])
            nc.sync.dma_start(out=out[:], in_=outt[:])

    nc.finalize()
    return nc


def _bd_host():
    bd = np.zeros((128, NCHUNK * IMGS), dtype=np.float32)
    for c in range(NCHUNK):
        for p in range(128):
            bd[p, c * IMGS + c * CHUNK_IMGS + p // RB] = 1.0
    return bd


def _tri_host():
    # tri[b, s] = 1 iff b <= s  (cumulative histogram)
    b = np.arange(NBINS)[:, None]
    s = np.arange(STEPS)[None, :]
    return (b <= s).astype(np.float32)


def kernel(x: np.ndarray) -> np.ndarray:
    assert x.shape == (B, C, H, W) and x.dtype == np.float32
    if "nc" not in _NC_CACHE:
        _NC_CACHE["nc"] = _build_nc()
    nc = _NC_CACHE["nc"]

    bd = _bd_host()
    tri = _tri_host()
    in_maps = []
    for i in range(NCORES):
        shard = x[i * (B // NCORES):(i + 1) * (B // NCORES)]  # (8, 3, 256, 256)
        in_maps.append({
            "x": np.ascontiguousarray(shard).reshape(NCHUNK * 128, FD),
            "bd": bd,
            "tri": tri,
        })
    res = run_bass_kernel_spmd(nc, in_maps, core_ids=list(range(NCORES)))
    parts = [res.results[i]["out"].T.reshape(B // NCORES, C, STEPS)
             for i in range(NCORES)]
    return np.concatenate(parts, axis=0).reshape(B, C * STEPS).astype(np.float32)


if __name__ == "__main__":
    rng = np.random.default_rng(0)
    x = rng.random((B, C, H, W), dtype=np.float32)
    y = kernel(x)
    print("kernel out", y.shape, y.dtype, y[:2, :6])


# revision 22
# speedup vs baseline: 1.8109x; 1.0479x over previous
"""Trainium2 Bass kernel for nn_CubECLayr: Euler characteristic curves of
sublevel cubical complexes, batch-data-parallel over 8 NeuronCores.

Algorithm (per core, 24 images of 256x256):
  1. k = ceil(x / DT) per pixel (exact integer bin), via fused multiply +
     magic-number round on the vector engine.  k in [1, 31] for x in (0,1).
  2. Vertex attribution: every cell (vertex/edge/square) of the cubical
     complex is anchored to its (value, index)-max vertex; the signed count
     of cells anchored at each pixel is an integer delta computed from
     neighbor comparisons in k-space.  Then ECC_s = sum_p delta_p * [k_p <= s].
  3. Histogram h[b] = sum_p delta_p * [k_p == b] via a CUSTOM DVE op that
     accumulates TWO radix-packed bins per pass:
         accum += delta * ([k==b0] + 2048*[k==b1])
     15 passes cover bins 1..30; bin 31 = sum(delta) - sum(bins 1..30),
     where sum(delta) rides the accumulator of the last delta-assembly op
     (tensor_tensor_reduce).  Fields are bounded by max|per-strip per-bin
     delta sum| = 161 << 1024, so the fp32 accumulator decodes exactly.
  4. Decode fields on-device (tiny vector ops), then two small PE matmuls:
     partition-reduce to per-image histograms and triangular-ones cumsum to
     the final ECC curves (output [32, 24], transposed on host).

Pipelining: kt is double-buffered; chunk c+1's bin-conversion and halo
shift-DMAs are issued between chunk c's delta assembly and its histogram
passes, so the halo DMA latency hides under the 15 histogram passes.

Layout: 3 chunks x 8 images; each image owns 16 partitions (16 rows each,
one halo row above/below via partition-shift SBUF DMAs; image-boundary
halo rows are PAD via two 8-partition memsets).
"""

from operator import add as _operator_add

import numpy as np

import concourse.bacc as bacc
import concourse.mybir as mybir
from concourse import tile
from concourse.bass_utils import run_bass_kernel_spmd

NCORES = 8
B, C, H, W = 64, 3, 256, 256
IMGS = (B // NCORES) * C          # 24 images per core
CHUNK_IMGS = 8
NCHUNK = IMGS // CHUNK_IMGS       # 3
RB = 16                           # partitions per image
ROWS = H // RB                    # 16 own rows per partition
FD = ROWS * W                     # 4096 own pixels per partition
STEPS = 32
NBINS = 33                        # bins 0..32 (k in [1,31]; 0/32 stay zero)
NPAIR = 15                        # hist passes: pairs (2g+1, 2g+2), bins 1..30
RADIX = 2048.0                    # field packing radix (fields |.| <= 161)
PAD = 1000.0                      # > any real bin; exact in fp16
MAGIC = 8388608.0                 # 2^23
HALF = float(np.float32(0.49999997))
F32 = mybir.dt.float32
F16 = mybir.dt.float16
Op = mybir.AluOpType
Ax = mybir.AxisListType

_NC_CACHE = {}
_HIST_OP_CACHE = {}


def _get_hist2_op():
    """Register (once per process) the custom DVE op
        out    = in1 * ([in0==s0] + imm2*[in0==s1])
        accum += sum(out)
    and return the DveOp handle."""
    if "op" in _HIST_OP_CACHE:
        return _HIST_OP_CACHE["op"]

    from concourse import dve_ops as dvo
    from concourse.dve_spec import Spec, Src0, Src1, C0, C1, C2, Zero, eq, lower
    from concourse.dve_uop import DveOpSpec

    name = "ECC_HIST2_ANT"

    def _ref(in0, in1, c0, c1, c2):
        a = in0.astype(np.float32)
        d = in1.astype(np.float32)
        body = (d * ((a == c0).astype(np.float32)
                     + c2 * (a == c1).astype(np.float32))).astype(np.float32)
        acc = body.reshape(body.shape[0], -1).sum(axis=-1, keepdims=True)
        return body, acc.astype(np.float32)

    spec = Spec(
        body=Src1 * (eq(Src0, C0) + C2 * eq(Src0, C1)),
        accum=_operator_add,
        accum_init=Zero,
        reference=_ref,
    )

    if name not in dvo._SUB_OPCODE_FOR_NAME:
        row = 1 + len(dvo.OPS)
        assert row < 0x20
        dvo._SUB_OPCODE_FOR_NAME[name] = row
        shas = {}
        for ver in ("v3", "v4"):
            sp = DveOpSpec(name=name, opcode=row, uops=lower(spec, ver=ver),
                           rd1_en=True)
            shas[ver] = sp.sha(ver)
        op = dvo.DveOp(name, spec, subdim=False, uops_sha=shas)
        dvo.OPS.append(op)
        dvo.CUSTOM_DVE_SPECS[name] = spec
    else:
        op = next(o for o in dvo.OPS if o.name == name)

    _HIST_OP_CACHE["op"] = op
    return op


def _build_nc():
    hist2 = _get_hist2_op()

    nc = bacc.Bacc(None, target_bir_lowering=False)
    x_in = nc.dram_tensor("x", [NCHUNK * 128, FD], F32, kind="ExternalInput")
    bd_in = nc.dram_tensor("bd", [128, NCHUNK * IMGS], F32, kind="ExternalInput")
    tri_in = nc.dram_tensor("tri", [NBINS, STEPS], F32, kind="ExternalInput")
    out = nc.dram_tensor("out", [STEPS, IMGS], F32, kind="ExternalOutput")

    with tile.TileContext(nc) as tc:
        with (
            tc.tile_pool(name="xp", bufs=2) as xp,
            tc.tile_pool(name="kp", bufs=2) as kp,
            tc.tile_pool(name="ap", bufs=2) as ap,
            tc.tile_pool(name="wp", bufs=1) as wp,
            tc.tile_pool(name="cst", bufs=1) as cst,
            tc.tile_pool(name="pp", bufs=1, space="PSUM") as pp,
            tc.tile_pool(name="pp2", bufs=1, space="PSUM") as pp2,
        ):
            bdt = cst.tile([128, NCHUNK * IMGS], F32)
            nc.sync.dma_start(out=bdt[:], in_=bd_in[:])
            trit = cst.tile([NBINS, STEPS], F32)
            nc.sync.dma_start(out=trit[:], in_=tri_in[:])
            padt = cst.tile([CHUNK_IMGS, W], F16)
            nc.vector.memset(padt[:], PAD)
            # per-chunk decoded histogram [128, 33]; cols 0 and 32 stay zero
            hist = cst.tile([128, NBINS], F32)
            nc.vector.memset(hist[:], 0.0)
            psum = pp.tile([NBINS, IMGS], F32)

            def conv_and_halo(c, split, on_dve):
                """DMA chunk c, convert to f16 bins kt (rows 1..16 at cols
                W..W+FD), fill halo rows: partition-shift DMAs for interior
                partitions, PAD overwrites at image boundaries.  For pipelined
                chunks (on_dve=False) the conversion runs on the otherwise-idle
                Scalar (affine) and GpSimd (magic round) engines."""
                kt = kp.tile([128, (ROWS + 2) * W], F16, tag="kt")
                xt = xp.tile([128, FD], F32, tag="xt")
                nfd = FD // split
                for h in range(split):
                    sl = slice(h * nfd, (h + 1) * nfd)
                    ksl = slice(W + h * nfd, W + (h + 1) * nfd)
                    nc.sync.dma_start(
                        out=xt[:, sl], in_=x_in[c * 128:(c + 1) * 128, sl])
                    if on_dve:
                        nc.vector.tensor_scalar(
                            out=xt[:, sl], in0=xt[:, sl], scalar1=31.0,
                            scalar2=HALF, op0=Op.mult, op1=Op.add)
                        nc.vector.tensor_scalar(
                            out=kt[:, ksl], in0=xt[:, sl], scalar1=MAGIC,
                            scalar2=-MAGIC, op0=Op.add, op1=Op.add)
                    else:
                        # same math on the otherwise-idle Scalar engine:
                        # y = 31x + HALF; y += MAGIC (fp32 rounds to int);
                        # kt = y - MAGIC (f16)
                        Act = mybir.ActivationFunctionType.Copy
                        nc.scalar.activation(
                            out=xt[:, sl], in_=xt[:, sl], func=Act,
                            bias=HALF, scale=31.0)
                        nc.scalar.activation(
                            out=xt[:, sl], in_=xt[:, sl], func=Act,
                            bias=MAGIC, scale=1.0)
                        nc.scalar.activation(
                            out=kt[:, ksl], in_=xt[:, sl], func=Act,
                            bias=-MAGIC, scale=1.0)
                # halo rows: top halo of partition p <- last own row of p-1,
                # bottom halo of p <- first own row of p+1; image-boundary
                # halo rows get PAD overwrites.
                nc.vector.memset(kt[:, 0:W], PAD)
                nc.vector.memset(kt[:, FD + W:FD + 2 * W], PAD)
                nc.gpsimd.dma_start(out=kt[1:128, 0:W], in_=kt[0:127, FD:FD + W])
                nc.gpsimd.dma_start(out=kt[0:127, FD + W:FD + 2 * W],
                                    in_=kt[1:128, W:2 * W])
                ktop = kt[:, 0:W].rearrange("(a b) w -> a b w", b=RB)
                nc.gpsimd.dma_start(out=ktop[:, 0, :], in_=padt[:])
                kbot = kt[:, FD + W:FD + 2 * W].rearrange("(a b) w -> a b w", b=RB)
                nc.gpsimd.dma_start(out=kbot[:, RB - 1, :], in_=padt[:])
                return kt

            def pre_assembly(kt, eng):
                """The four kt-neighborhood compares; engine-parametric so
                pipelined chunks can run them on GpSimd under the previous
                chunk's histogram passes."""
                # rh[r, j] = [k(r, j+1) >= k(r, j)], own rows, j = 0..254
                # (col 255 crosses rows; harmless, later masked via t zeroing)
                rh = ap.tile([128, FD], F16, tag="rh")
                eng.tensor_tensor(
                    out=rh[:], in0=kt[:, W + 1:W + FD + 1], in1=kt[:, W:W + FD],
                    op=Op.is_ge)
                # rv[t, j] = [k(row t+1) >= k(row t)], t = 0..16 (17 rows)
                rv = ap.tile([128, (ROWS + 1) * W], F16, tag="rv")
                eng.tensor_tensor(
                    out=rv[:], in0=kt[:, W:], in1=kt[:, 0:(ROWS + 1) * W],
                    op=Op.is_ge)
                # khe[r, j] = max(k(r, j), k(r, j+1)), rows 0..17
                khe = ap.tile([128, (ROWS + 2) * W], F16, tag="khe")
                eng.tensor_tensor(
                    out=khe[:, 0:(ROWS + 2) * W - 1],
                    in0=kt[:, 0:(ROWS + 2) * W - 1], in1=kt[:, 1:(ROWS + 2) * W],
                    op=Op.max)
                eng.memset(khe[:, (ROWS + 2) * W - 1:(ROWS + 2) * W], PAD)
                # u[t, j] = [khe(row t+1, j) >= khe(row t, j)], t = 0..16
                ut = ap.tile([128, (ROWS + 1) * W], F16, tag="ut")
                eng.tensor_tensor(
                    out=ut[:], in0=khe[:, W:], in1=khe[:, 0:(ROWS + 1) * W],
                    op=Op.is_ge)
                return rh, rv, ut  # noqa: eng is always nc.vector today

            kts = [None] * NCHUNK
            pre = [None] * NCHUNK
            kts[0] = conv_and_halo(0, split=2, on_dve=True)
            pre[0] = pre_assembly(kts[0], nc.vector)

            for c in range(NCHUNK):
                kt = kts[c]
                rh, rv, ut = pre[c]

                # Cc[r, j] = u(r) - u(r-1) for own rows r (u rows 1..16 - 0..15)
                cc = wp.tile([128, FD], F16, tag="cc")
                nc.vector.tensor_tensor(
                    out=cc[:], in0=ut[:, W:], in1=ut[:, 0:FD], op=Op.subtract)
                # zero col 255 of each row (cross-row garbage in rh/cc)
                cc3 = cc[:].rearrange("p (r w) -> p r w", w=W)
                nc.vector.memset(cc3[:, :, W - 1:W], 0.0)
                # t = rh * Cc
                tt = wp.tile([128, FD], F16, tag="tt")
                nc.vector.tensor_tensor(out=tt[:], in0=rh[:], in1=cc[:], op=Op.mult)

                # --- delta assembly ---
                # delta = rv(below) - rv(above) + t - shift1(t) - Cc
                dl = wp.tile([128, FD], F16, tag="dl")
                nc.vector.tensor_tensor(
                    out=dl[:], in0=rv[:, W:], in1=rv[:, 0:FD], op=Op.subtract)
                nc.vector.tensor_tensor(out=dl[:], in0=dl[:], in1=tt[:], op=Op.add)
                nc.vector.tensor_tensor(
                    out=dl[:, 1:FD], in0=dl[:, 1:FD], in1=tt[:, 0:FD - 1],
                    op=Op.subtract)
                # final: dl -= Cc (2x TT); tot = sum(delta) via the Scalar
                # engine's activation accumulator, off the Vector critical path
                nc.vector.tensor_tensor(
                    out=dl[:], in0=dl[:], in1=cc[:], op=Op.subtract)
                tot = wp.tile([128, 1], F32, tag="tot")
                wmb = wp.tile([128, FD], F16, tag="wmb")
                nc.scalar.activation(
                    out=wmb[:], in_=dl[:],
                    func=mybir.ActivationFunctionType.Copy,
                    bias=0.0, scale=1.0, accum_out=tot[:])

                # pipeline: issue next chunk's convert + halo DMAs + neighbor
                # compares now; they run on Scalar/GpSimd under this chunk's
                # histogram passes
                if c + 1 < NCHUNK:
                    kts[c + 1] = conv_and_halo(c + 1, split=1, on_dve=False)
                    pre[c + 1] = pre_assembly(kts[c + 1], nc.vector)

                # --- 15 packed-histogram passes:
                #     acc[:, g] = sum(delta * ([k==2g+1] + RADIX*[k==2g+2])) ---
                acc = wp.tile([128, NPAIR], F32, tag="acc")
                wm = wp.tile([128, FD], F16, tag="wm")
                for g in range(NPAIR):
                    nc.vector._custom_dve(
                        hist2,
                        out=wm[:],
                        in0=kt[:, W:W + FD],
                        in1=dl[:],
                        s0=float(2 * g + 1),
                        s1=float(2 * g + 2),
                        imm2=RADIX,
                        accum_out=acc[:, g:g + 1],
                    )

                # --- decode packed fields into hist[:, 1..31] ---
                # hi = round(acc / RADIX); lo = acc - RADIX*hi
                dec = wp.tile([128, NPAIR], F32, tag="dec")
                nc.vector.tensor_scalar(
                    out=dec[:], in0=acc[:], scalar1=1.0 / RADIX, scalar2=MAGIC,
                    op0=Op.mult, op1=Op.add)
                # hi -> even bins 2,4,...,30
                hist_hi = hist[:, 2:STEPS].rearrange("p (g two) -> p g two", two=2)
                nc.vector.tensor_scalar(
                    out=hist_hi[:, :, 0], in0=dec[:], scalar1=-MAGIC, scalar2=0.0,
                    op0=Op.add, op1=Op.add)
                # lo = acc - RADIX*hi -> odd bins 1,3,...,29
                hist_lo = hist[:, 1:STEPS - 1].rearrange("p (g two) -> p g two", two=2)
                nc.vector.scalar_tensor_tensor(
                    out=hist_lo[:, :, 0], in0=hist_hi[:, :, 0], scalar=-RADIX,
                    in1=acc[:], op0=Op.mult, op1=Op.add)
                # bin 31 = tot - sum(bins 1..30)
                s30 = wp.tile([128, 1], F32, tag="s30")
                nc.vector.tensor_reduce(
                    out=s30[:], in_=hist[:, 1:STEPS - 1], axis=Ax.X, op=Op.add)
                nc.vector.tensor_tensor(
                    out=hist[:, STEPS - 1:STEPS], in0=tot[:], in1=s30[:],
                    op=Op.subtract)

                # --- partition partials -> per-bin-per-image (PSUM accumulate) ---
                nc.tensor.matmul(
                    psum[:], hist[:], bdt[:, c * IMGS:(c + 1) * IMGS],
                    start=(c == 0), stop=(c == NCHUNK - 1))

            # --- cumulative sum over bins via triangular-ones matmul ---
            h2 = cst.tile([NBINS, IMGS], F32)
            nc.vector.tensor_copy(out=h2[:], in_=psum[:])
            psum3 = pp2.tile([STEPS, IMGS], F32)
            nc.tensor.matmul(psum3[:], trit[:], h2[:], start=True, stop=True)
            outt = cst.tile([STEPS, IMGS], F32)
            nc.vector.tensor_copy(out=outt[:], in_=psum3[:])
            nc.sync.dma_start(out=out[:], in_=outt[:])

    nc.finalize()
    return nc


def _bd_host():
    bd = np.zeros((128, NCHUNK * IMGS), dtype=np.float32)
    for c in range(NCHUNK):
        for p in range(128):
            bd[p, c * IMGS + c * CHUNK_IMGS + p // RB] = 1.0
    return bd


def _tri_host():
    # tri[b, s] = 1 iff b <= s  (cumulative histogram)
    b = np.arange(NBINS)[:, None]
    s = np.arange(STEPS)[None, :]
    return (b <= s).astype(np.float32)


def kernel(x: np.ndarray) -> np.ndarray:
    assert x.shape == (B, C, H, W) and x.dtype == np.float32
    if "nc" not in _NC_CACHE:
        _NC_CACHE["nc"] = _build_nc()
    nc = _NC_CACHE["nc"]

    bd = _bd_host()
    tri = _tri_host()
    in_maps = []
    for i in range(NCORES):
        shard = x[i * (B // NCORES):(i + 1) * (B // NCORES)]  # (8, 3, 256, 256)
        in_maps.append({
            "x": np.ascontiguousarray(shard).reshape(NCHUNK * 128, FD),
            "bd": bd,
            "tri": tri,
        })
    res = run_bass_kernel_spmd(nc, in_maps, core_ids=list(range(NCORES)))
    parts = [res.results[i]["out"].T.reshape(B // NCORES, C, STEPS)
             for i in range(NCORES)]
    return np.concatenate(parts, axis=0).reshape(B, C * STEPS).astype(np.float32)


if __name__ == "__main__":
    rng = np.random.default_rng(0)
    x = rng.random((B, C, H, W), dtype=np.float32)
    y = kernel(x)
    print("kernel out", y.shape, y.dtype, y[:2, :6])


# revision 28
# speedup vs baseline: 1.8725x; 1.0340x over previous
"""Trainium2 Bass kernel for nn_CubECLayr: Euler characteristic curves of
sublevel cubical complexes, batch-data-parallel over 8 NeuronCores.

Algorithm (per core, 24 images of 256x256):
  1. k = ceil(x / DT) per pixel (exact integer bin), via fused multiply +
     magic-number round on the vector engine.  k in [1, 31] for x in (0,1).
  2. Vertex attribution: every cell (vertex/edge/square) of the cubical
     complex is anchored to its (value, index)-max vertex; the signed count
     of cells anchored at each pixel is an integer delta computed from
     neighbor comparisons in k-space.  Then ECC_s = sum_p delta_p * [k_p <= s].
  3. Histogram h[b] = sum_p delta_p * [k_p == b] via a CUSTOM DVE op that
     accumulates TWO radix-packed bins per pass:
         accum += delta * ([k==b0] + 2048*[k==b1])
     15 passes cover bins 1..30; bin 31 = sum(delta) - sum(bins 1..30),
     where sum(delta) rides the accumulator of the last delta-assembly op
     (tensor_tensor_reduce).  Fields are bounded by max|per-strip per-bin
     delta sum| = 161 << 1024, so the fp32 accumulator decodes exactly.
  4. Decode fields on-device (tiny vector ops), then two small PE matmuls:
     partition-reduce to per-image histograms and triangular-ones cumsum to
     the final ECC curves (output [32, 24], transposed on host).

Pipelining: kt is double-buffered; chunk c+1's bin-conversion and halo
shift-DMAs are issued between chunk c's delta assembly and its histogram
passes, so the halo DMA latency hides under the 15 histogram passes.

Layout: 3 chunks x 8 images; each image owns 16 partitions (16 rows each,
one halo row above/below via partition-shift SBUF DMAs; image-boundary
halo rows are PAD via two 8-partition memsets).
"""

from operator import add as _operator_add

import numpy as np

import concourse.bacc as bacc
import concourse.mybir as mybir
from concourse import tile
from concourse.bass_utils import run_bass_kernel_spmd

NCORES = 8
B, C, H, W = 64, 3, 256, 256
IMGS = (B // NCORES) * C          # 24 images per core
CHUNK_IMGS = 8
NCHUNK = IMGS // CHUNK_IMGS       # 3
RB = 16                           # partitions per image
ROWS = H // RB                    # 16 own rows per partition
FD = ROWS * W                     # 4096 own pixels per partition
STEPS = 32
NBINS = 33                        # bins 0..32 (k in [1,31]; 0/32 stay zero)
NPAIR = 15                        # hist passes: pairs (2g+1, 2g+2), bins 1..30
RADIX = 2048.0                    # field packing radix (fields |.| <= 161)
PAD = 1000.0                      # > any real bin; exact in fp16
MAGIC = 8388608.0                 # 2^23
HALF = float(np.float32(0.49999997))
F32 = mybir.dt.float32
F16 = mybir.dt.float16
Op = mybir.AluOpType
Ax = mybir.AxisListType

_NC_CACHE = {}
_HIST_OP_CACHE = {}


def _get_hist2_op():
    """Register (once per process) the custom DVE op
        out    = in1 * ([in0==s0] + imm2*[in0==s1])
        accum += sum(out)
    and return the DveOp handle."""
    if "op" in _HIST_OP_CACHE:
        return _HIST_OP_CACHE["op"]

    from concourse import dve_ops as dvo
    from concourse.dve_spec import Spec, Src0, Src1, C0, C1, C2, Zero, eq, lower
    from concourse.dve_uop import DveOpSpec

    name = "ECC_HIST2_ANT"

    def _ref(in0, in1, c0, c1, c2):
        a = in0.astype(np.float32)
        d = in1.astype(np.float32)
        body = (d * ((a == c0).astype(np.float32)
                     + c2 * (a == c1).astype(np.float32))).astype(np.float32)
        acc = body.reshape(body.shape[0], -1).sum(axis=-1, keepdims=True)
        return body, acc.astype(np.float32)

    spec = Spec(
        body=Src1 * (eq(Src0, C0) + C2 * eq(Src0, C1)),
        accum=_operator_add,
        accum_init=Zero,
        reference=_ref,
    )

    if name not in dvo._SUB_OPCODE_FOR_NAME:
        row = 1 + len(dvo.OPS)
        assert row < 0x20
        dvo._SUB_OPCODE_FOR_NAME[name] = row
        shas = {}
        for ver in ("v3", "v4"):
            sp = DveOpSpec(name=name, opcode=row, uops=lower(spec, ver=ver),
                           rd1_en=True)
            shas[ver] = sp.sha(ver)
        op = dvo.DveOp(name, spec, subdim=False, uops_sha=shas)
        dvo.OPS.append(op)
        dvo.CUSTOM_DVE_SPECS[name] = spec
    else:
        op = next(o for o in dvo.OPS if o.name == name)

    _HIST_OP_CACHE["op"] = op
    return op


def _build_nc():
    hist2 = _get_hist2_op()

    nc = bacc.Bacc(None, target_bir_lowering=False)
    x_in = nc.dram_tensor("x", [NCHUNK * 128, FD], F32, kind="ExternalInput")
    bd_in = nc.dram_tensor("bd", [128, NCHUNK * IMGS], F32, kind="ExternalInput")
    tri_in = nc.dram_tensor("tri", [NBINS, STEPS], F32, kind="ExternalInput")
    out = nc.dram_tensor("out", [STEPS, IMGS], F32, kind="ExternalOutput")

    with tile.TileContext(nc) as tc:
        with (
            tc.tile_pool(name="xp", bufs=2) as xp,
            tc.tile_pool(name="kp", bufs=2) as kp,
            tc.tile_pool(name="ap", bufs=2) as ap,
            tc.tile_pool(name="wp", bufs=1) as wp,
            tc.tile_pool(name="cst", bufs=1) as cst,
            tc.tile_pool(name="pp", bufs=1, space="PSUM") as pp,
            tc.tile_pool(name="pp2", bufs=1, space="PSUM") as pp2,
        ):
            bdt = cst.tile([128, NCHUNK * IMGS], F32)
            nc.sync.dma_start(out=bdt[:], in_=bd_in[:])
            trit = cst.tile([NBINS, STEPS], F32)
            nc.sync.dma_start(out=trit[:], in_=tri_in[:])
            padt = cst.tile([CHUNK_IMGS, W], F16)
            nc.vector.memset(padt[:], PAD)
            # per-chunk decoded histogram [128, 33]; cols 0 and 32 stay zero
            hist = cst.tile([128, NBINS], F32)
            nc.vector.memset(hist[:], 0.0)
            psum = pp.tile([NBINS, IMGS], F32)

            def conv_and_halo(c, split, on_dve):
                """DMA chunk c, convert to f16 bins kt (rows 1..16 at cols
                W..W+FD).  Halo rows are loaded straight from DRAM (the same
                pixel rows the neighboring partition owns) and converted too;
                image-boundary halo rows get PAD via two tiny gpsimd memsets.
                For pipelined chunks (on_dve=False) all conversion runs on the
                otherwise-idle Scalar engine."""
                kt = kp.tile([128, (ROWS + 2) * W], F16, tag="kt")
                xt = xp.tile([128, FD], F32, tag="xt")
                xh = xp.tile([128, 2 * W], F32, tag="xh")
                # halo sources: top halo of p = last row of p-1, bottom halo
                # of p = first row of p+1 (partition-shifted DRAM reads).
                # Edge partitions read a dummy valid row; they are image
                # boundaries and get PAD-overwritten below.
                if c == 0:
                    nc.sync.dma_start(
                        out=xh[0:1, 0:W], in_=x_in[0:1, FD - W:FD])
                    nc.sync.dma_start(
                        out=xh[1:128, 0:W], in_=x_in[0:127, FD - W:FD])
                else:
                    nc.sync.dma_start(
                        out=xh[:, 0:W],
                        in_=x_in[c * 128 - 1:c * 128 + 127, FD - W:FD])
                if c == NCHUNK - 1:
                    nc.sync.dma_start(
                        out=xh[0:127, W:2 * W],
                        in_=x_in[c * 128 + 1:c * 128 + 128, 0:W])
                    nc.sync.dma_start(
                        out=xh[127:128, W:2 * W],
                        in_=x_in[c * 128 + 127:c * 128 + 128, 0:W])
                else:
                    nc.sync.dma_start(
                        out=xh[:, W:2 * W],
                        in_=x_in[c * 128 + 1:c * 128 + 129, 0:W])
                Act = mybir.ActivationFunctionType.Copy

                def affine(out_, in_):
                    if on_dve:
                        nc.vector.tensor_scalar(
                            out=out_, in0=in_, scalar1=31.0, scalar2=HALF,
                            op0=Op.mult, op1=Op.add)
                    else:
                        nc.scalar.activation(out=out_, in_=in_, func=Act,
                                             bias=HALF, scale=31.0)

                def roundto(out_, in_):
                    if on_dve:
                        nc.vector.tensor_scalar(
                            out=out_, in0=in_, scalar1=MAGIC, scalar2=-MAGIC,
                            op0=Op.add, op1=Op.add)
                    else:
                        nc.scalar.activation(out=in_, in_=in_, func=Act,
                                             bias=MAGIC, scale=1.0)
                        nc.scalar.activation(out=out_, in_=in_, func=Act,
                                             bias=-MAGIC, scale=1.0)

                nfd = FD // split
                for h in range(split):
                    sl = slice(h * nfd, (h + 1) * nfd)
                    ksl = slice(W + h * nfd, W + (h + 1) * nfd)
                    nc.sync.dma_start(
                        out=xt[:, sl], in_=x_in[c * 128:(c + 1) * 128, sl])
                    affine(xt[:, sl], xt[:, sl])
                    roundto(kt[:, ksl], xt[:, sl])
                # halo conversion (full width; edge rows are dummies)
                affine(xh[:, :], xh[:, :])
                roundto(kt[:, 0:W], xh[:, 0:W])
                roundto(kt[:, FD + W:FD + 2 * W], xh[:, W:2 * W])
                # image-boundary halo rows: PAD
                ktop = kt[:, 0:W].rearrange("(a b) w -> a b w", b=RB)
                nc.gpsimd.dma_start(out=ktop[:, 0, :], in_=padt[:])
                kbot = kt[:, FD + W:FD + 2 * W].rearrange("(a b) w -> a b w", b=RB)
                nc.gpsimd.dma_start(out=kbot[:, RB - 1, :], in_=padt[:])
                return kt

            def pre_assembly(kt, eng):
                """The four kt-neighborhood compares; engine-parametric so
                pipelined chunks can run them on GpSimd under the previous
                chunk's histogram passes."""
                # rh[r, j] = [k(r, j+1) >= k(r, j)], own rows, j = 0..254
                # (col 255 crosses rows; harmless, later masked via t zeroing)
                rh = ap.tile([128, FD], F16, tag="rh")
                eng.tensor_tensor(
                    out=rh[:], in0=kt[:, W + 1:W + FD + 1], in1=kt[:, W:W + FD],
                    op=Op.is_ge)
                # rv[t, j] = [k(row t+1) >= k(row t)], t = 0..16 (17 rows)
                rv = ap.tile([128, (ROWS + 1) * W], F16, tag="rv")
                eng.tensor_tensor(
                    out=rv[:], in0=kt[:, W:], in1=kt[:, 0:(ROWS + 1) * W],
                    op=Op.is_ge)
                # khe[r, j] = max(k(r, j), k(r, j+1)), rows 0..17
                khe = ap.tile([128, (ROWS + 2) * W], F16, tag="khe")
                eng.tensor_tensor(
                    out=khe[:, 0:(ROWS + 2) * W - 1],
                    in0=kt[:, 0:(ROWS + 2) * W - 1], in1=kt[:, 1:(ROWS + 2) * W],
                    op=Op.max)
                eng.memset(khe[:, (ROWS + 2) * W - 1:(ROWS + 2) * W], PAD)
                # u[t, j] = [khe(row t+1, j) >= khe(row t, j)], t = 0..16
                ut = ap.tile([128, (ROWS + 1) * W], F16, tag="ut")
                eng.tensor_tensor(
                    out=ut[:], in0=khe[:, W:], in1=khe[:, 0:(ROWS + 1) * W],
                    op=Op.is_ge)
                return rh, rv, ut  # noqa: eng is always nc.vector today

            kts = [None] * NCHUNK
            pre = [None] * NCHUNK
            kts[0] = conv_and_halo(0, split=2, on_dve=True)
            pre[0] = pre_assembly(kts[0], nc.vector)

            for c in range(NCHUNK):
                kt = kts[c]
                rh, rv, ut = pre[c]

                # Cc[r, j] = u(r) - u(r-1) for own rows r (u rows 1..16 - 0..15)
                cc = wp.tile([128, FD], F16, tag="cc")
                nc.vector.tensor_tensor(
                    out=cc[:], in0=ut[:, W:], in1=ut[:, 0:FD], op=Op.subtract)
                # zero col 255 of each row (cross-row garbage in rh/cc)
                cc3 = cc[:].rearrange("p (r w) -> p r w", w=W)
                nc.vector.memset(cc3[:, :, W - 1:W], 0.0)
                # t = rh * Cc
                tt = wp.tile([128, FD], F16, tag="tt")
                nc.vector.tensor_tensor(out=tt[:], in0=rh[:], in1=cc[:], op=Op.mult)

                # --- delta assembly ---
                # delta = rv(below) - rv(above) + t - shift1(t) - Cc
                dl = wp.tile([128, FD], F16, tag="dl")
                nc.vector.tensor_tensor(
                    out=dl[:], in0=rv[:, W:], in1=rv[:, 0:FD], op=Op.subtract)
                nc.vector.tensor_tensor(out=dl[:], in0=dl[:], in1=tt[:], op=Op.add)
                nc.vector.tensor_tensor(
                    out=dl[:, 1:FD], in0=dl[:, 1:FD], in1=tt[:, 0:FD - 1],
                    op=Op.subtract)
                # final: dl -= Cc (2x TT); tot = sum(delta) via the Scalar
                # engine's activation accumulator, off the Vector critical path
                nc.vector.tensor_tensor(
                    out=dl[:], in0=dl[:], in1=cc[:], op=Op.subtract)
                tot = wp.tile([128, 1], F32, tag="tot")
                wmb = wp.tile([128, FD], F16, tag="wmb")
                nc.scalar.activation(
                    out=wmb[:], in_=dl[:],
                    func=mybir.ActivationFunctionType.Copy,
                    bias=0.0, scale=1.0, accum_out=tot[:])

                # pipeline: issue next chunk's convert + halo DMAs + neighbor
                # compares now; they run on Scalar/GpSimd under this chunk's
                # histogram passes
                if c + 1 < NCHUNK:
                    kts[c + 1] = conv_and_halo(c + 1, split=1, on_dve=False)
                    pre[c + 1] = pre_assembly(kts[c + 1], nc.vector)

                # --- 15 packed-histogram passes:
                #     acc[:, g] = sum(delta * ([k==2g+1] + RADIX*[k==2g+2])) ---
                acc = wp.tile([128, NPAIR], F32, tag="acc")
                wm = wp.tile([128, FD], F16, tag="wm")
                for g in range(NPAIR):
                    nc.vector._custom_dve(
                        hist2,
                        out=wm[:],
                        in0=kt[:, W:W + FD],
                        in1=dl[:],
                        s0=float(2 * g + 1),
                        s1=float(2 * g + 2),
                        imm2=RADIX,
                        accum_out=acc[:, g:g + 1],
                    )

                # --- decode packed fields into hist[:, 1..31] ---
                # hi = round(acc / RADIX); lo = acc - RADIX*hi
                dec = wp.tile([128, NPAIR], F32, tag="dec")
                nc.vector.tensor_scalar(
                    out=dec[:], in0=acc[:], scalar1=1.0 / RADIX, scalar2=MAGIC,
                    op0=Op.mult, op1=Op.add)
                # hi -> even bins 2,4,...,30
                hist_hi = hist[:, 2:STEPS].rearrange("p (g two) -> p g two", two=2)
                nc.vector.tensor_scalar(
                    out=hist_hi[:, :, 0], in0=dec[:], scalar1=-MAGIC, scalar2=0.0,
                    op0=Op.add, op1=Op.add)
                # lo = acc - RADIX*hi -> odd bins 1,3,...,29
                hist_lo = hist[:, 1:STEPS - 1].rearrange("p (g two) -> p g two", two=2)
                nc.vector.scalar_tensor_tensor(
                    out=hist_lo[:, :, 0], in0=hist_hi[:, :, 0], scalar=-RADIX,
                    in1=acc[:], op0=Op.mult, op1=Op.add)
                # bin 31 = tot - sum(bins 1..30)
                s30 = wp.tile([128, 1], F32, tag="s30")
                nc.vector.tensor_reduce(
                    out=s30[:], in_=hist[:, 1:STEPS - 1], axis=Ax.X, op=Op.add)
                nc.vector.tensor_tensor(
                    out=hist[:, STEPS - 1:STEPS], in0=tot[:], in1=s30[:],
                    op=Op.subtract)

                # --- partition partials -> per-bin-per-image (PSUM accumulate) ---
                nc.tensor.matmul(
                    psum[:], hist[:], bdt[:, c * IMGS:(c + 1) * IMGS],
                    start=(c == 0), stop=(c == NCHUNK - 1))

            # --- cumulative sum over bins via triangular-ones matmul ---
            h2 = cst.tile([NBINS, IMGS], F32)
            nc.vector.tensor_copy(out=h2[:], in_=psum[:])
            psum3 = pp2.tile([STEPS, IMGS], F32)
            nc.tensor.matmul(psum3[:], trit[:], h2[:], start=True, stop=True)
            outt = cst.tile([STEPS, IMGS], F32)
            nc.vector.tensor_copy(out=outt[:], in_=psum3[:])
            nc.sync.dma_start(out=out[:], in_=outt[:])

    nc.finalize()
    return nc


def _bd_host():
    bd = np.zeros((128, NCHUNK * IMGS), dtype=np.float32)
    for c in range(NCHUNK):
        for p in range(128):
            bd[p, c * IMGS + c * CHUNK_IMGS + p // RB] = 1.0
    return bd


def _tri_host():
    # tri[b, s] = 1 iff b <= s  (cumulative histogram)
    b = np.arange(NBINS)[:, None]
    s = np.arange(STEPS)[None, :]
    return (b <= s).astype(np.float32)


def kernel(x: np.ndarray) -> np.ndarray:
    assert x.shape == (B, C, H, W) and x.dtype == np.float32
    if "nc" not in _NC_CACHE:
        _NC_CACHE["nc"] = _build_nc()
    nc = _NC_CACHE["nc"]

    bd = _bd_host()
    tri = _tri_host()
    in_maps = []
    for i in range(NCORES):
        shard = x[i * (B // NCORES):(i + 1) * (B // NCORES)]  # (8, 3, 256, 256)
        in_maps.append({
            "x": np.ascontiguousarray(shard).reshape(NCHUNK * 128, FD),
            "bd": bd,
            "tri": tri,
        })
    res = run_bass_kernel_spmd(nc, in_maps, core_ids=list(range(NCORES)))
    parts = [res.results[i]["out"].T.reshape(B // NCORES, C, STEPS)
             for i in range(NCORES)]
    return np.concatenate(parts, axis=0).reshape(B, C * STEPS).astype(np.float32)


if __name__ == "__main__":
    rng = np.random.default_rng(0)
    x = rng.random((B, C, H, W), dtype=np.float32)
    y = kernel(x)
    print("kernel out", y.shape, y.dtype, y[:2, :6])
